# revision 19
# baseline (speedup 1.0000x reference)
"""Sliding-window GQA attention (B=2,T=2048,D=2048,N=8,K=4,H=256,W=1024) on 8 trn2 cores.

Sharding: batch over 2 (fsdp) x heads over 4 (tp). Core (b, tp) computes 2 q heads /
1 kv head for batch b; partial [T, D] outputs are summed over tp on the host.

All matmuls are bf16 x bf16 (measured to match f32r's ~0.43ns/row when the PE stays
busy, with half-size LDWEIGHTS that always hide) accumulating in f32 PSUM; bf16
ap-128 stays full-rate, which lets phase B skip the guaranteed-masked halves of the
two window-edge key blocks. Inputs ship bf16, host-packed so every DMA moves
contiguous >=2KB rows: x halves on the SP ring, weights/tables on the ACT ring in
first-use order.

Per-core device pipeline:
  A: per 512-token quarter, k and both-q projections interleaved per d-chunk (so
     quarter 0 paces with the x-half DMA stream); v in natural [t,h] layout from
     bf16 x-chunk stationaries, emitted between the rope emissions so PE never
     waits on ACT/DVE; fused RMS-norm (sum-of-squares via all-ones matmul, rsqrt
     via ACT spline) + RoPE (host sin/cos tables) out of PSUM; rope work is split
     ACT (squares, rsqrt), GpSimd (sin/cos scaling), DVE (rotation), and the
     PSUM->SBUF v copies ride ACT.
  B: per 256-token query pair: logits^T = kT^T qT per 128-key block (window blocks
     only; edge blocks only on their live query half), exp on ACT (no max
     subtraction: |logit| <= 16), triangular masks on GpSimd, denominator + P^T V
     via PE accumulation, divide via Square(rsqrt) on ACT + one DVE mult. Pairs run
     [1..7, 0] so the cheap pair lands last.
  C: out = pvT^T o_w accumulated over local heads, emitted between a pair's logits
     and tail so its matmuls fill exp-wait windows; PSUM->SBUF copies on DVE.
"""
import os

import numpy as np
import ml_dtypes

import concourse.bacc as bacc
import concourse.mybir as mybir
from concourse.tile import TileContext
from concourse.bass_utils import run_bass_kernel_spmd

try:  # pragma: no cover - profiling hook is optional
    from antenv.axon_hooks import get_axon_ntff_profile_hook  # noqa: F401
except ImportError:
    os.environ.setdefault("BASS_NEVER_TRACE", "1")


F32 = mybir.dt.float32
F32R = mybir.dt.float32r
BF16 = mybir.dt.bfloat16
AF = mybir.ActivationFunctionType
OP = mybir.AluOpType

B, T, D = 2, 2048, 2048
N, KV, H = 8, 4, 256
WINDOW = 1024
BASE_FREQ = 10000.0
EPS = 1e-6
NB = T // 128          # 16 token blocks
NQ = 4                 # t quarters for projections (512 each)
NPAIR = 8              # query-block pairs (256 tokens each)
NPB = np.dtype(ml_dtypes.bfloat16)


def _mask_idx(i, j):
    if j == i + 1:
        return 3
    if j == i:
        return 2
    if j == i - 7:
        return 1
    if j == i - 8:
        return 0
    return None


def _jlist(i):
    return list(range(max(0, i - 8), i + 2))


def _build():
    nc = bacc.Bacc(None)

    xh = nc.dram_tensor("xh", [2, 16, 128, 1024], BF16, kind="ExternalInput")
    # fused k/q weights, d-major pairs: [c, 128, 2, 3, 256]; per-d [kw|qw_nl0|qw_nl1]
    wA = nc.dram_tensor("wA", [8, 128, 2, 3, 256], BF16, kind="ExternalInput")
    vw = nc.dram_tensor("vw", [128, 16, 256], BF16, kind="ExternalInput")
    ow = nc.dram_tensor("ow", [2, 2, 128, D], BF16, kind="ExternalInput")  # [nl,hh,128,D]
    cosT = nc.dram_tensor("cosT", [4, 128, 512], F32, kind="ExternalInput")
    sinT = nc.dram_tensor("sinT", [4, 128, 512], F32, kind="ExternalInput")
    masks = nc.dram_tensor("masks", [128, 4, 256], BF16, kind="ExternalInput")
    scs = nc.dram_tensor("scs", [128, 2, 2], F32, kind="ExternalInput")  # (1+scale)[q/k][hh]
    out = nc.dram_tensor("out", [16, 128, D], BF16, kind="ExternalOutput")

    with TileContext(nc) as tc:
        with tc.tile_pool(name="pers", bufs=1) as pers:
            kT_sb = pers.tile([128, 2, T], BF16)
            v_sb = pers.tile([128, NB, H], BF16)
            qT_sb = pers.tile([128, 2, 2, T], BF16)
            scs_sb = pers.tile([128, 2, 2], F32)
            ones32 = pers.tile([128, 128], F32)
            ones_b = pers.tile([128, 128], BF16)
            bias_q = pers.tile([128, 1], F32)
            bias_k = pers.tile([128, 1], F32)
            bias_z = pers.tile([128, 1], F32)
            masks_sb = pers.tile([128, 4, 256], BF16)
            ow_sb = pers.tile([128, 2, 2, D], BF16)

            nc.vector.memset(ones32, 1.0)
            nc.vector.tensor_copy(ones_b, ones32)
            nc.vector.memset(bias_q, float(H * EPS))
            nc.vector.memset(bias_k, EPS)
            nc.vector.memset(bias_z, 0.0)

            # ---------------- Phase A: all projections + rms + rope ----------------
            with tc.tile_pool(name="wts", bufs=1) as wts, \
                 tc.tile_pool(name="xs", bufs=20) as xs, \
                 tc.tile_pool(name="ropep", bufs=1) as ropep, \
                 tc.tile_pool(name="psA", bufs=1, space="PSUM") as psum:

                # ACT-ring DMAs in first-use order (x rides the SP ring).
                wA_t = {}
                for c in range(8):
                    wA_t[c] = wts.tile([128, 2, 3, 256], BF16, tag=f"wA{c}", name=f"wA{c}")
                    nc.scalar.dma_start(out=wA_t[c], in_=wA[c, :, :, :, :])
                vw_sb = wts.tile([128, 16, 256], BF16)
                nc.scalar.dma_start(out=vw_sb, in_=vw[:, :, :])
                cs_t, ss_t = {}, {}
                for qt in range(NQ):
                    cs_t[qt] = ropep.tile([128, 512], F32, tag=f"cst{qt}", name=f"cst{qt}")
                    ss_t[qt] = ropep.tile([128, 512], F32, tag=f"sst{qt}", name=f"sst{qt}")
                    nc.scalar.dma_start(out=cs_t[qt], in_=cosT[qt, :, :])
                    nc.scalar.dma_start(out=ss_t[qt], in_=sinT[qt, :, :])
                    if qt == 0:
                        nc.scalar.dma_start(out=scs_sb, in_=scs[:, :, :])
                        nc.scalar.dma_start(out=masks_sb, in_=masks[:, :, :])
                for nl in range(2):
                    for hh in range(2):
                        nc.scalar.dma_start(out=ow_sb[:, nl, hh, :], in_=ow[nl, hh, :, :])

                # x halves on the SP ring
                xh_t = {}
                for half in range(2):
                    for d in range(16):
                        xt = xs.tile([128, 1024], BF16, tag="xt")
                        nc.sync.dma_start(out=xt, in_=xh[half, d, :, :])
                        xh_t[(half, d)] = xt

                def rope_emit(p0, p1, dst, kind, qt, eng=None):
                    # p0/p1: [128, 512] psum (raw proj h-halves); dst: [128, 2, 512].
                    # In the last quarter the psum is first staged to SBUF (eng) so
                    # phase B's PSUM pool doesn't wait on the rotation chain.
                    sq0 = ropep.tile([128, 512], BF16, tag="sq0", bufs=2)
                    sq1 = ropep.tile([128, 512], BF16, tag="sq1", bufs=2)
                    nc.scalar.activation(sq0, p0, AF.Square)
                    nc.scalar.activation(sq1, p1, AF.Square)
                    pss = psum.tile([128, 512], F32, tag="pss", bufs=1)
                    nc.tensor.matmul(pss, ones_b, sq0, start=True, stop=False)
                    nc.tensor.matmul(pss, ones_b, sq1, start=False, stop=True)
                    if eng is not None:
                        pc0 = ropep.tile([128, 512], F32, tag="pc0", bufs=3)
                        pc1 = ropep.tile([128, 512], F32, tag="pc1", bufs=3)
                        if eng == "act":
                            nc.scalar.copy(pc0, p0)
                            nc.scalar.copy(pc1, p1)
                        else:
                            nc.vector.tensor_copy(pc0, p0)
                            nc.vector.tensor_copy(pc1, p1)
                        p0, p1 = pc0, pc1
                    rs = ropep.tile([128, 512], F32, tag="rs", bufs=2)
                    if kind == "q":
                        # 1/16 * rsqrt(ss/256 + eps) == 1/sqrt(ss + 256*eps)
                        nc.scalar.activation(rs, pss, AF.Abs_reciprocal_sqrt,
                                             scale=1.0, bias=bias_q)
                    else:
                        nc.scalar.activation(rs, pss, AF.Abs_reciprocal_sqrt,
                                             scale=1.0 / H, bias=bias_k)
                    cs = ropep.tile([128, 512], F32, tag="cs", bufs=2)
                    ss = ropep.tile([128, 512], F32, tag="ss", bufs=2)
                    nc.gpsimd.tensor_tensor(cs, cs_t[qt], rs, OP.mult)
                    nc.gpsimd.tensor_tensor(ss, ss_t[qt], rs, OP.mult)
                    ki = 0 if kind == "q" else 1
                    s0 = scs_sb[:, ki, 0:1]
                    s1 = scs_sb[:, ki, 1:2]
                    t0 = ropep.tile([128, 512], F32, tag="t0", bufs=2)
                    t1 = ropep.tile([128, 512], F32, tag="t1", bufs=2)
                    nc.vector.scalar_tensor_tensor(t0, p0, s0, cs, OP.mult, OP.mult)
                    nc.vector.scalar_tensor_tensor(t1, p1, s1, ss, OP.mult, OP.mult)
                    nc.vector.tensor_tensor(dst[:, 0, :], t0, t1, OP.subtract)
                    t2 = ropep.tile([128, 512], F32, tag="t0", bufs=2)
                    t3 = ropep.tile([128, 512], F32, tag="t1", bufs=2)
                    nc.vector.scalar_tensor_tensor(t2, p1, s1, cs, OP.mult, OP.mult)
                    nc.vector.scalar_tensor_tensor(t3, p0, s0, ss, OP.mult, OP.mult)
                    nc.vector.tensor_tensor(dst[:, 1, :], t2, t3, OP.add)

                for qt in range(NQ):
                    tq = slice(512 * qt, 512 * (qt + 1))
                    half, qo = qt // 2, (qt % 2) * 512
                    xts = [xh_t[(half, d)][:, qo:qo + 512] for d in range(16)]
                    # k + both-q interleaved per d-chunk (quarter 0 paces with DMAs)
                    pk = [psum.tile([128, 512], F32, tag=f"pk{hh}", bufs=1,
                                    name=f"pk{qt}_{hh}") for hh in range(2)]
                    pq = [[psum.tile([128, 512], F32, tag=f"pq{nl}{hh}", bufs=1,
                                     name=f"pq{qt}_{nl}{hh}") for hh in range(2)]
                          for nl in range(2)]
                    # quarter 0 interleaves k+q per d-chunk to pace with the x DMA
                    # stream; later quarters run the k pass first so the previous
                    # quarter's q-rope chains can drain their PSUM banks before the
                    # q pass needs them (no staging copies required).
                    if qt == 0:
                        for d in range(16):
                            w = wA_t[d // 2][:, d % 2]
                            st, sp = d == 0, d == 15
                            nc.tensor.matmul(pk[0], w[:, 0, 0:128], xts[d],
                                             start=st, stop=sp)
                            nc.tensor.matmul(pk[1], w[:, 0, 128:256], xts[d],
                                             start=st, stop=sp)
                            for nl in range(2):
                                nc.tensor.matmul(pq[nl][0], w[:, 1 + nl, 0:128], xts[d],
                                                 start=st, stop=sp)
                                nc.tensor.matmul(pq[nl][1], w[:, 1 + nl, 128:256], xts[d],
                                                 start=st, stop=sp)
                    else:
                        for d in range(16):
                            w = wA_t[d // 2][:, d % 2]
                            st, sp = d == 0, d == 15
                            nc.tensor.matmul(pk[0], w[:, 0, 0:128], xts[d],
                                             start=st, stop=sp)
                            nc.tensor.matmul(pk[1], w[:, 0, 128:256], xts[d],
                                             start=st, stop=sp)
                        for d in range(16):
                            w = wA_t[d // 2][:, d % 2]
                            st, sp = d == 0, d == 15
                            for nl in range(2):
                                nc.tensor.matmul(pq[nl][0], w[:, 1 + nl, 0:128], xts[d],
                                                 start=st, stop=sp)
                                nc.tensor.matmul(pq[nl][1], w[:, 1 + nl, 128:256], xts[d],
                                                 start=st, stop=sp)

                    # v natural [t,h]: bf16 x-chunk stationary, vw moving; the v
                    # matmuls give ACT time to square pk/pq before the pss mms.
                    def v_half(hf):
                        pvv = psum.tile([128, 2, H], F32, tag="pva", bufs=1,
                                        name=f"pvv{qt}_{hf}")
                        tc0 = 4 * qt + 2 * hf
                        for sub in range(2):
                            tl = slice(128 * (2 * hf + sub), 128 * (2 * hf + sub) + 128)
                            for d in range(16):
                                nc.tensor.matmul(pvv[:, sub, :], xts[d][:, tl],
                                                 vw_sb[:, d, :],
                                                 start=(d == 0), stop=(d == 15))
                        nc.scalar.copy(v_sb[:, tc0:tc0 + 2, :], pvv)
                    last = qt == NQ - 1
                    v_half(0)
                    rope_emit(pk[0], pk[1], kT_sb[:, :, tq], "k", qt,
                              eng="act" if last else None)
                    v_half(1)
                    rope_emit(pq[0][0], pq[0][1], qT_sb[:, 0, :, tq], "q", qt,
                              eng="dve" if last else None)
                    rope_emit(pq[1][0], pq[1][1], qT_sb[:, 1, :, tq], "q", qt,
                              eng="act" if last else None)

            # ---------------- Phases B + C ----------------
            with tc.tile_pool(name="persB", bufs=1) as persB, \
                 tc.tile_pool(name="expt", bufs=14) as expt, \
                 tc.tile_pool(name="bw", bufs=4) as bw, \
                 tc.tile_pool(name="oc", bufs=3) as oc, \
                 tc.tile_pool(name="psB", bufs=1, space="PSUM") as psumB:

                pvT_sb = persB.tile([128, 2, 2, T], BF16)

                def _span(i, j):
                    # query half actually alive for this key block (128-granular)
                    if j == i + 1:
                        return 128, 256
                    if j == i - 8:
                        return 0, 128
                    return 0, 256

                def emit_logits_exp(pi):
                    i = 2 * pi
                    q0 = 256 * pi
                    js = _jlist(i)
                    ets = {}
                    for nl in range(2):
                        for k in range(0, len(js), 2):
                            jp = js[k:k + 2]
                            lp = psumB.tile([128, 2, 256], F32, tag="lp", bufs=4)
                            spans = [_span(i, j) for j in jp]
                            for x2, j in enumerate(jp):
                                lo, hi = spans[x2]
                                sj = slice(128 * j, 128 * (j + 1))
                                nc.tensor.matmul(lp[:, x2, lo:hi], kT_sb[:, 0, sj],
                                                 qT_sb[:, nl, 0, q0 + lo:q0 + hi],
                                                 start=True, stop=False)
                                nc.tensor.matmul(lp[:, x2, lo:hi], kT_sb[:, 1, sj],
                                                 qT_sb[:, nl, 1, q0 + lo:q0 + hi],
                                                 start=False, stop=True)
                            et = expt.tile([128, 2, 256], BF16, tag="et")
                            if spans == [(0, 256), (0, 256)]:
                                nc.scalar.activation(et, lp, AF.Exp)
                            else:
                                for x2, (lo, hi) in enumerate(spans):
                                    nc.scalar.activation(et[:, x2, lo:hi],
                                                         lp[:, x2, lo:hi], AF.Exp)
                            for x2, j in enumerate(jp):
                                lo, hi = spans[x2]
                                mi = _mask_idx(i, j)
                                if mi is not None:
                                    m = masks_sb[:, mi, lo:hi]
                                    nc.gpsimd.tensor_tensor(et[:, x2, lo:hi],
                                                            et[:, x2, lo:hi], m, OP.mult)
                                ets[(nl, j)] = (et[:, x2, lo:hi], lo, hi)
                    return ets

                def emit_tail(pi, ets):
                    i = 2 * pi
                    tqs = slice(256 * pi, 256 * (pi + 1))
                    js = _jlist(i)
                    for nl in range(2):
                        pd = psumB.tile([128, 256], F32, tag="pd", bufs=1)
                        for idx, j in enumerate(js):
                            ap, lo, hi = ets[(nl, j)]
                            nc.tensor.matmul(pd[:, lo:hi], ones_b, ap,
                                             start=(idx == 0), stop=(idx == len(js) - 1),
                                             skip_group_check=True)
                        # 1/den = Square(rsqrt(den)): two cheap ACT splines, one DVE mult
                        r1 = bw.tile([128, 256], F32, tag="r1")
                        nc.scalar.activation(r1, pd, AF.Abs_reciprocal_sqrt,
                                             scale=1.0, bias=bias_z)
                        r2 = bw.tile([128, 256], F32, tag="r2")
                        nc.scalar.activation(r2, r1, AF.Square)
                        for hh in range(2):
                            pv = psumB.tile([128, 256], F32, tag="pvb", bufs=3)
                            hs = slice(128 * hh, 128 * (hh + 1))
                            for idx, j in enumerate(js):
                                ap, lo, hi = ets[(nl, j)]
                                nc.tensor.matmul(pv[:, lo:hi], v_sb[:, j, hs], ap,
                                                 start=(idx == 0),
                                                 stop=(idx == len(js) - 1),
                                                 skip_group_check=True)
                            nc.vector.tensor_tensor(pvT_sb[:, nl, hh, tqs], pv, r2, OP.mult)

                def emit_oproj(pi):
                    # output projection for this pair's two token blocks
                    for tb in (2 * pi, 2 * pi + 1):
                        ts_ = slice(128 * tb, 128 * (tb + 1))
                        od = oc.tile([128, D], BF16, tag="od", bufs=3)
                        for dt in range(4):
                            dsl = slice(512 * dt, 512 * (dt + 1))
                            po = psumB.tile([128, 512], F32, tag="pvb", bufs=3)
                            step = 0
                            for nl in range(2):
                                for hh in range(2):
                                    nc.tensor.matmul(po, pvT_sb[:, nl, hh, ts_],
                                                     ow_sb[:, nl, hh, dsl],
                                                     start=(step == 0), stop=(step == 3))
                                    step += 1
                            nc.vector.tensor_copy(od[:, dsl], po)
                        nc.sync.dma_start(out=out[tb, :, :], in_=od)

                # pairs in [1..7, 0]: the cheap pair lands last; each pair's o-proj
                # is emitted between the NEXT pair's logits and tail so its matmuls
                # fill the exp-wait windows.
                order = list(range(1, NPAIR)) + [0]
                prev = None
                for pi in order:
                    ets = emit_logits_exp(pi)
                    if prev is not None:
                        emit_oproj(prev)
                    emit_tail(pi, ets)
                    prev = pi
                emit_oproj(0)

    nc.compile()
    return nc


_prog = None
last_results = None


def kernel(x, positions, q_w, k_w, v_w, o_w, q_norm_scale, k_norm_scale):
    global _prog, last_results
    x = np.asarray(x); positions = np.asarray(positions)
    q_w = np.asarray(q_w); k_w = np.asarray(k_w); v_w = np.asarray(v_w); o_w = np.asarray(o_w)
    q_norm_scale = np.asarray(q_norm_scale); k_norm_scale = np.asarray(k_norm_scale)

    if _prog is None:
        _prog = _build()
    nc = _prog

    # host-side constants
    j = np.arange(H // 2, dtype=np.float32)
    timescale = (BASE_FREQ ** (2.0 / H * j)).astype(np.float32)

    c = np.arange(128)[:, None]
    r = np.arange(128)[None, :]
    up = (c <= r).astype(np.float32)
    lo = (c > r).astype(np.float32)
    one_b = np.ones((128, 128), np.float32)
    zero_b = np.zeros((128, 128), np.float32)
    masks_np = np.stack([
        np.concatenate([lo, zero_b], 1),
        np.concatenate([one_b, lo], 1),
        np.concatenate([up, one_b], 1),
        np.concatenate([zero_b, up], 1),
    ], axis=0).transpose(1, 0, 2).astype(NPB)  # [128, 4, 256]

    scs_np = np.empty((128, 2, 2), np.float32)
    scs_np[:, 0, 0] = 1.0 + q_norm_scale[:128]
    scs_np[:, 0, 1] = 1.0 + q_norm_scale[128:]
    scs_np[:, 1, 0] = 1.0 + k_norm_scale[:128]
    scs_np[:, 1, 1] = 1.0 + k_norm_scale[128:]

    in_maps = []
    for core in range(8):
        b, tp = core // 4, core % 4
        sinu = positions[b].astype(np.float32)[:, None] / timescale[None, :]  # [T, 128]
        cos_np = np.cos(sinu).T.reshape(128, 4, 512).transpose(1, 0, 2)  # [4,128,512]
        sin_np = np.sin(sinu).T.reshape(128, 4, 512).transpose(1, 0, 2)
        # x^T packed [half, d, 128, 1024]
        xT = x[b].T.reshape(16, 128, 2, 1024).transpose(2, 0, 1, 3)
        # fused k/q weights [c, 128, 2, 3, 256]
        wA_np = np.empty((16, 128, 3, 256), np.float32)
        kw = k_w[tp]; qw = q_w[2 * tp:2 * tp + 2]
        wA_np[:, :, 0, :] = kw.reshape(16, 128, 256)
        wA_np[:, :, 1, :] = qw[0].reshape(16, 128, 256)
        wA_np[:, :, 2, :] = qw[1].reshape(16, 128, 256)
        wA_np = wA_np.reshape(8, 2, 128, 3, 256).transpose(0, 2, 1, 3, 4)
        ow_np = o_w[2 * tp:2 * tp + 2].reshape(2, 2, 128, D)  # [nl, hh, 128, D]
        in_maps.append({
            "xh": np.ascontiguousarray(xT).astype(NPB),
            "wA": np.ascontiguousarray(wA_np).astype(NPB),
            "vw": np.ascontiguousarray(
                v_w[tp].reshape(16, 128, 256).transpose(1, 0, 2)).astype(NPB),
            "ow": np.ascontiguousarray(ow_np).astype(NPB),
            "cosT": np.ascontiguousarray(cos_np),
            "sinT": np.ascontiguousarray(sin_np),
            "masks": masks_np,
            "scs": scs_np,
        })

    res = run_bass_kernel_spmd(nc, in_maps, core_ids=list(range(8)))
    last_results = res

    out = np.zeros((B, T, D), np.float32)
    for core in range(8):
        out[core // 4] += np.asarray(res.results[core]["out"]).astype(np.float32).reshape(T, D)
    return out


# revision 21
# speedup vs baseline: 1.0128x; 1.0128x over previous
"""Sliding-window GQA attention (B=2,T=2048,D=2048,N=8,K=4,H=256,W=1024) on 8 trn2 cores.

Sharding: batch over 2 (fsdp) x heads over 4 (tp). Core (b, tp) computes 2 q heads /
1 kv head for batch b; partial [T, D] outputs are summed over tp on the host.

All matmuls are bf16 x bf16 (measured to match f32r's ~0.43ns/row when the PE stays
busy, with half-size LDWEIGHTS that always hide) accumulating in f32 PSUM; bf16
ap-128 stays full-rate, which lets phase B skip the guaranteed-masked halves of the
two window-edge key blocks. Inputs ship bf16, host-packed so every DMA moves
contiguous >=2KB rows: x halves on the SP ring, weights/tables on the ACT ring in
first-use order.

Per-core device pipeline:
  A: per 512-token quarter, k and both-q projections interleaved per d-chunk (so
     quarter 0 paces with the x-half DMA stream); v in natural [t,h] layout from
     bf16 x-chunk stationaries, emitted between the rope emissions so PE never
     waits on ACT/DVE; fused RMS-norm (sum-of-squares via all-ones matmul, rsqrt
     via ACT spline) + RoPE (host sin/cos tables) out of PSUM; rope work is split
     ACT (squares, rsqrt), GpSimd (sin/cos scaling), DVE (rotation), and the
     PSUM->SBUF v copies ride ACT.
  B: per 256-token query pair: logits^T = kT^T qT per 128-key block (window blocks
     only; edge blocks only on their live query half), exp on ACT (no max
     subtraction: |logit| <= 16), triangular masks on GpSimd, denominator + P^T V
     via PE accumulation, divide via Square(rsqrt) on ACT + one DVE mult. Pairs run
     [1..7, 0] so the cheap pair lands last.
  C: out = pvT^T o_w accumulated over local heads, emitted between a pair's logits
     and tail so its matmuls fill exp-wait windows; PSUM->SBUF copies on DVE.
"""
import os

import numpy as np
import ml_dtypes

import concourse.bacc as bacc
import concourse.mybir as mybir
from concourse.tile import TileContext
from concourse.bass_utils import run_bass_kernel_spmd

try:  # pragma: no cover - profiling hook is optional
    from antenv.axon_hooks import get_axon_ntff_profile_hook  # noqa: F401
except ImportError:
    os.environ.setdefault("BASS_NEVER_TRACE", "1")


F32 = mybir.dt.float32
F32R = mybir.dt.float32r
BF16 = mybir.dt.bfloat16
AF = mybir.ActivationFunctionType
OP = mybir.AluOpType

B, T, D = 2, 2048, 2048
N, KV, H = 8, 4, 256
WINDOW = 1024
BASE_FREQ = 10000.0
EPS = 1e-6
NB = T // 128          # 16 token blocks
NQ = 4                 # t quarters for projections (512 each)
NPAIR = 8              # query-block pairs (256 tokens each)
NPB = np.dtype(ml_dtypes.bfloat16)


def _mask_idx(i, j):
    if j == i + 1:
        return 3
    if j == i:
        return 2
    if j == i - 7:
        return 1
    if j == i - 8:
        return 0
    return None


def _jlist(i):
    return list(range(max(0, i - 8), i + 2))


def _build():
    nc = bacc.Bacc(None)

    xh = nc.dram_tensor("xh", [2, 16, 128, 1024], BF16, kind="ExternalInput")
    # fused k/q weights, d-major pairs: [c, 128, 2, 3, 256]; per-d [kw|qw_nl0|qw_nl1]
    wA = nc.dram_tensor("wA", [8, 128, 2, 3, 256], BF16, kind="ExternalInput")
    vw = nc.dram_tensor("vw", [128, 16, 256], BF16, kind="ExternalInput")
    ow = nc.dram_tensor("ow", [2, 2, 128, D], BF16, kind="ExternalInput")  # [nl,hh,128,D]
    cosT = nc.dram_tensor("cosT", [4, 128, 512], F32, kind="ExternalInput")
    sinT = nc.dram_tensor("sinT", [4, 128, 512], F32, kind="ExternalInput")
    masks = nc.dram_tensor("masks", [128, 4, 256], BF16, kind="ExternalInput")
    scs = nc.dram_tensor("scs", [128, 2, 2], F32, kind="ExternalInput")  # (1+scale)[q/k][hh]
    out = nc.dram_tensor("out", [16, 128, D], BF16, kind="ExternalOutput")

    with TileContext(nc) as tc:
        with tc.tile_pool(name="pers", bufs=1) as pers:
            kT_sb = pers.tile([128, 2, T], BF16)
            v_sb = pers.tile([128, NB, H], BF16)
            qT_sb = pers.tile([128, 2, 2, T], BF16)
            scs_sb = pers.tile([128, 2, 2], F32)
            ones32 = pers.tile([128, 128], F32)
            ones_b = pers.tile([128, 128], BF16)
            bias_q = pers.tile([128, 1], F32)
            bias_k = pers.tile([128, 1], F32)
            bias_z = pers.tile([128, 1], F32)
            masks_sb = pers.tile([128, 4, 256], BF16)
            ow_sb = pers.tile([128, 2, 2, D], BF16)

            nc.vector.memset(ones32, 1.0)
            nc.vector.tensor_copy(ones_b, ones32)
            nc.vector.memset(bias_q, float(H * EPS))
            nc.vector.memset(bias_k, EPS)
            nc.vector.memset(bias_z, 0.0)

            # ---------------- Phase A: all projections + rms + rope ----------------
            with tc.tile_pool(name="wts", bufs=1) as wts, \
                 tc.tile_pool(name="xs", bufs=20) as xs, \
                 tc.tile_pool(name="ropep", bufs=1) as ropep, \
                 tc.tile_pool(name="psA", bufs=1, space="PSUM") as psum:

                # ACT-ring DMAs in first-use order (x rides the SP ring).
                wA_t = {}
                for c in range(8):
                    wA_t[c] = wts.tile([128, 2, 3, 256], BF16, tag=f"wA{c}", name=f"wA{c}")
                    nc.scalar.dma_start(out=wA_t[c], in_=wA[c, :, :, :, :])
                vw_sb = wts.tile([128, 16, 256], BF16)
                nc.scalar.dma_start(out=vw_sb, in_=vw[:, :, :])
                cs_t, ss_t = {}, {}
                for qt in range(NQ):
                    cs_t[qt] = ropep.tile([128, 512], F32, tag=f"cst{qt}", name=f"cst{qt}")
                    ss_t[qt] = ropep.tile([128, 512], F32, tag=f"sst{qt}", name=f"sst{qt}")
                    nc.scalar.dma_start(out=cs_t[qt], in_=cosT[qt, :, :])
                    nc.scalar.dma_start(out=ss_t[qt], in_=sinT[qt, :, :])
                    if qt == 0:
                        nc.scalar.dma_start(out=scs_sb, in_=scs[:, :, :])
                        nc.scalar.dma_start(out=masks_sb, in_=masks[:, :, :])
                for nl in range(2):
                    for hh in range(2):
                        nc.scalar.dma_start(out=ow_sb[:, nl, hh, :], in_=ow[nl, hh, :, :])

                # x halves on the SP ring
                xh_t = {}
                for half in range(2):
                    for d in range(16):
                        xt = xs.tile([128, 1024], BF16, tag="xt")
                        nc.sync.dma_start(out=xt, in_=xh[half, d, :, :])
                        xh_t[(half, d)] = xt

                def rope_sq(p0, p1):
                    # squares on ACT; emitted right after the producing pass so
                    # they run while PE continues with the next matmul pass
                    sq0 = ropep.tile([128, 512], BF16, tag="sq0", bufs=2)
                    sq1 = ropep.tile([128, 512], BF16, tag="sq1", bufs=2)
                    nc.scalar.activation(sq0, p0, AF.Square)
                    nc.scalar.activation(sq1, p1, AF.Square)
                    return sq0, sq1

                def rope_rest(sq01, p0, p1, dst, kind, qt, eng=None):
                    # p0/p1: [128, 512] psum (raw proj h-halves); dst: [128, 2, 512].
                    # In the last quarter the psum is first staged to SBUF (eng) so
                    # phase B's PSUM pool doesn't wait on the rotation chain.
                    sq0, sq1 = sq01
                    pss = psum.tile([128, 512], F32, tag="pss", bufs=1)
                    nc.tensor.matmul(pss, ones_b, sq0, start=True, stop=False)
                    nc.tensor.matmul(pss, ones_b, sq1, start=False, stop=True)
                    if eng is not None:
                        pc0 = ropep.tile([128, 512], F32, tag="pc0", bufs=3)
                        pc1 = ropep.tile([128, 512], F32, tag="pc1", bufs=3)
                        if eng == "act":
                            nc.scalar.copy(pc0, p0)
                            nc.scalar.copy(pc1, p1)
                        else:
                            nc.vector.tensor_copy(pc0, p0)
                            nc.vector.tensor_copy(pc1, p1)
                        p0, p1 = pc0, pc1
                    rs = ropep.tile([128, 512], F32, tag="rs", bufs=2)
                    if kind == "q":
                        # 1/16 * rsqrt(ss/256 + eps) == 1/sqrt(ss + 256*eps)
                        nc.scalar.activation(rs, pss, AF.Abs_reciprocal_sqrt,
                                             scale=1.0, bias=bias_q)
                    else:
                        nc.scalar.activation(rs, pss, AF.Abs_reciprocal_sqrt,
                                             scale=1.0 / H, bias=bias_k)
                    cs = ropep.tile([128, 512], F32, tag="cs", bufs=2)
                    ss = ropep.tile([128, 512], F32, tag="ss", bufs=2)
                    nc.gpsimd.tensor_tensor(cs, cs_t[qt], rs, OP.mult)
                    nc.gpsimd.tensor_tensor(ss, ss_t[qt], rs, OP.mult)
                    ki = 0 if kind == "q" else 1
                    s0 = scs_sb[:, ki, 0:1]
                    s1 = scs_sb[:, ki, 1:2]
                    t0 = ropep.tile([128, 512], F32, tag="t0", bufs=2)
                    t1 = ropep.tile([128, 512], F32, tag="t1", bufs=2)
                    nc.vector.scalar_tensor_tensor(t0, p0, s0, cs, OP.mult, OP.mult)
                    nc.vector.scalar_tensor_tensor(t1, p1, s1, ss, OP.mult, OP.mult)
                    nc.vector.tensor_tensor(dst[:, 0, :], t0, t1, OP.subtract)
                    t2 = ropep.tile([128, 512], F32, tag="t0", bufs=2)
                    t3 = ropep.tile([128, 512], F32, tag="t1", bufs=2)
                    nc.vector.scalar_tensor_tensor(t2, p1, s1, cs, OP.mult, OP.mult)
                    nc.vector.scalar_tensor_tensor(t3, p0, s0, ss, OP.mult, OP.mult)
                    nc.vector.tensor_tensor(dst[:, 1, :], t2, t3, OP.add)

                for qt in range(NQ):
                    tq = slice(512 * qt, 512 * (qt + 1))
                    half, qo = qt // 2, (qt % 2) * 512
                    xts = [xh_t[(half, d)][:, qo:qo + 512] for d in range(16)]
                    # k + both-q interleaved per d-chunk (quarter 0 paces with DMAs)
                    pk = [psum.tile([128, 512], F32, tag=f"pk{hh}", bufs=1,
                                    name=f"pk{qt}_{hh}") for hh in range(2)]
                    pq = [[psum.tile([128, 512], F32, tag=f"pq{nl}{hh}", bufs=1,
                                     name=f"pq{qt}_{nl}{hh}") for hh in range(2)]
                          for nl in range(2)]
                    # quarter 0 interleaves k+q per d-chunk to pace with the x DMA
                    # stream; later quarters run the k pass first and its rope right
                    # after, so each rope chain drains its PSUM banks a full pass
                    # before the next quarter needs them.
                    def k_mm(d):
                        w = wA_t[d // 2][:, d % 2]
                        st, sp = d == 0, d == 15
                        nc.tensor.matmul(pk[0], w[:, 0, 0:128], xts[d],
                                         start=st, stop=sp)
                        nc.tensor.matmul(pk[1], w[:, 0, 128:256], xts[d],
                                         start=st, stop=sp)

                    def q_mm(d):
                        w = wA_t[d // 2][:, d % 2]
                        st, sp = d == 0, d == 15
                        for nl in range(2):
                            nc.tensor.matmul(pq[nl][0], w[:, 1 + nl, 0:128], xts[d],
                                             start=st, stop=sp)
                            nc.tensor.matmul(pq[nl][1], w[:, 1 + nl, 128:256], xts[d],
                                             start=st, stop=sp)

                    # v natural [t,h]: bf16 x-chunk stationary, vw moving; the v
                    # matmuls give ACT time to square pk/pq before the pss mms.
                    def v_half(hf):
                        pvv = psum.tile([128, 2, H], F32, tag="pva", bufs=1,
                                        name=f"pvv{qt}_{hf}")
                        tc0 = 4 * qt + 2 * hf
                        for sub in range(2):
                            tl = slice(128 * (2 * hf + sub), 128 * (2 * hf + sub) + 128)
                            for d in range(16):
                                nc.tensor.matmul(pvv[:, sub, :], xts[d][:, tl],
                                                 vw_sb[:, d, :],
                                                 start=(d == 0), stop=(d == 15))
                        nc.scalar.copy(v_sb[:, tc0:tc0 + 2, :], pvv)

                    last = qt == NQ - 1
                    kdst = kT_sb[:, :, tq]
                    if qt == 0:
                        for d in range(16):
                            k_mm(d)
                            q_mm(d)
                        sqk = rope_sq(pk[0], pk[1])
                        v_half(0)
                        rope_rest(sqk, pk[0], pk[1], kdst, "k", qt)
                        sq0 = rope_sq(pq[0][0], pq[0][1])
                        v_half(1)
                        rope_rest(sq0, pq[0][0], pq[0][1], qT_sb[:, 0, :, tq], "q", qt)
                        sq1 = rope_sq(pq[1][0], pq[1][1])
                        rope_rest(sq1, pq[1][0], pq[1][1], qT_sb[:, 1, :, tq], "q", qt)
                    else:
                        for d in range(16):
                            k_mm(d)
                        sqk = rope_sq(pk[0], pk[1])
                        for d in range(16):
                            q_mm(d)
                        rope_rest(sqk, pk[0], pk[1], kdst, "k", qt)
                        sq0 = rope_sq(pq[0][0], pq[0][1])
                        v_half(0)
                        rope_rest(sq0, pq[0][0], pq[0][1], qT_sb[:, 0, :, tq], "q", qt,
                                  eng="dve" if last else None)
                        sq1 = rope_sq(pq[1][0], pq[1][1])
                        v_half(1)
                        rope_rest(sq1, pq[1][0], pq[1][1], qT_sb[:, 1, :, tq], "q", qt,
                                  eng="act" if last else None)

            # ---------------- Phases B + C ----------------
            with tc.tile_pool(name="persB", bufs=1) as persB, \
                 tc.tile_pool(name="expt", bufs=14) as expt, \
                 tc.tile_pool(name="bw", bufs=4) as bw, \
                 tc.tile_pool(name="oc", bufs=3) as oc, \
                 tc.tile_pool(name="psB", bufs=1, space="PSUM") as psumB:

                pvT_sb = persB.tile([128, 2, 2, T], BF16)

                def _span(i, j):
                    # query half actually alive for this key block (128-granular)
                    if j == i + 1:
                        return 128, 256
                    if j == i - 8:
                        return 0, 128
                    return 0, 256

                def emit_logits_exp(pi):
                    i = 2 * pi
                    q0 = 256 * pi
                    js = _jlist(i)
                    ets = {}
                    for nl in range(2):
                        for k in range(0, len(js), 2):
                            jp = js[k:k + 2]
                            lp = psumB.tile([128, 2, 256], F32, tag="lp", bufs=4)
                            spans = [_span(i, j) for j in jp]
                            for x2, j in enumerate(jp):
                                lo, hi = spans[x2]
                                sj = slice(128 * j, 128 * (j + 1))
                                nc.tensor.matmul(lp[:, x2, lo:hi], kT_sb[:, 0, sj],
                                                 qT_sb[:, nl, 0, q0 + lo:q0 + hi],
                                                 start=True, stop=False)
                                nc.tensor.matmul(lp[:, x2, lo:hi], kT_sb[:, 1, sj],
                                                 qT_sb[:, nl, 1, q0 + lo:q0 + hi],
                                                 start=False, stop=True)
                            et = expt.tile([128, 2, 256], BF16, tag="et")
                            if spans == [(0, 256), (0, 256)]:
                                nc.scalar.activation(et, lp, AF.Exp)
                            else:
                                for x2, (lo, hi) in enumerate(spans):
                                    nc.scalar.activation(et[:, x2, lo:hi],
                                                         lp[:, x2, lo:hi], AF.Exp)
                            for x2, j in enumerate(jp):
                                lo, hi = spans[x2]
                                mi = _mask_idx(i, j)
                                if mi is not None:
                                    m = masks_sb[:, mi, lo:hi]
                                    nc.gpsimd.tensor_tensor(et[:, x2, lo:hi],
                                                            et[:, x2, lo:hi], m, OP.mult)
                                ets[(nl, j)] = (et[:, x2, lo:hi], lo, hi)
                    return ets

                def emit_tail(pi, ets):
                    i = 2 * pi
                    tqs = slice(256 * pi, 256 * (pi + 1))
                    js = _jlist(i)
                    for nl in range(2):
                        pd = psumB.tile([128, 256], F32, tag="pd", bufs=1)
                        for idx, j in enumerate(js):
                            ap, lo, hi = ets[(nl, j)]
                            nc.tensor.matmul(pd[:, lo:hi], ones_b, ap,
                                             start=(idx == 0), stop=(idx == len(js) - 1),
                                             skip_group_check=True)
                        # 1/den = Square(rsqrt(den)): two cheap ACT splines, one DVE mult
                        r1 = bw.tile([128, 256], F32, tag="r1")
                        nc.scalar.activation(r1, pd, AF.Abs_reciprocal_sqrt,
                                             scale=1.0, bias=bias_z)
                        r2 = bw.tile([128, 256], F32, tag="r2")
                        nc.scalar.activation(r2, r1, AF.Square)
                        for hh in range(2):
                            pv = psumB.tile([128, 256], F32, tag="pvb", bufs=3)
                            hs = slice(128 * hh, 128 * (hh + 1))
                            for idx, j in enumerate(js):
                                ap, lo, hi = ets[(nl, j)]
                                nc.tensor.matmul(pv[:, lo:hi], v_sb[:, j, hs], ap,
                                                 start=(idx == 0),
                                                 stop=(idx == len(js) - 1),
                                                 skip_group_check=True)
                            nc.vector.tensor_tensor(pvT_sb[:, nl, hh, tqs], pv, r2, OP.mult)

                def emit_oproj(pi):
                    # output projection for this pair's two token blocks
                    for tb in (2 * pi, 2 * pi + 1):
                        ts_ = slice(128 * tb, 128 * (tb + 1))
                        od = oc.tile([128, D], BF16, tag="od", bufs=3)
                        for dt in range(4):
                            dsl = slice(512 * dt, 512 * (dt + 1))
                            po = psumB.tile([128, 512], F32, tag="pvb", bufs=3)
                            step = 0
                            for nl in range(2):
                                for hh in range(2):
                                    nc.tensor.matmul(po, pvT_sb[:, nl, hh, ts_],
                                                     ow_sb[:, nl, hh, dsl],
                                                     start=(step == 0), stop=(step == 3))
                                    step += 1
                            nc.vector.tensor_copy(od[:, dsl], po)
                        nc.sync.dma_start(out=out[tb, :, :], in_=od)

                # pairs in [1..7, 0]: the cheap pair lands last; each pair's o-proj
                # is emitted between the NEXT pair's logits and tail so its matmuls
                # fill the exp-wait windows.
                order = list(range(1, NPAIR)) + [0]
                prev = None
                for pi in order:
                    ets = emit_logits_exp(pi)
                    if prev is not None:
                        emit_oproj(prev)
                    emit_tail(pi, ets)
                    prev = pi
                emit_oproj(0)

    nc.compile()
    return nc


_prog = None
last_results = None


def kernel(x, positions, q_w, k_w, v_w, o_w, q_norm_scale, k_norm_scale):
    global _prog, last_results
    x = np.asarray(x); positions = np.asarray(positions)
    q_w = np.asarray(q_w); k_w = np.asarray(k_w); v_w = np.asarray(v_w); o_w = np.asarray(o_w)
    q_norm_scale = np.asarray(q_norm_scale); k_norm_scale = np.asarray(k_norm_scale)

    if _prog is None:
        _prog = _build()
    nc = _prog

    # host-side constants
    j = np.arange(H // 2, dtype=np.float32)
    timescale = (BASE_FREQ ** (2.0 / H * j)).astype(np.float32)

    c = np.arange(128)[:, None]
    r = np.arange(128)[None, :]
    up = (c <= r).astype(np.float32)
    lo = (c > r).astype(np.float32)
    one_b = np.ones((128, 128), np.float32)
    zero_b = np.zeros((128, 128), np.float32)
    masks_np = np.stack([
        np.concatenate([lo, zero_b], 1),
        np.concatenate([one_b, lo], 1),
        np.concatenate([up, one_b], 1),
        np.concatenate([zero_b, up], 1),
    ], axis=0).transpose(1, 0, 2).astype(NPB)  # [128, 4, 256]

    scs_np = np.empty((128, 2, 2), np.float32)
    scs_np[:, 0, 0] = 1.0 + q_norm_scale[:128]
    scs_np[:, 0, 1] = 1.0 + q_norm_scale[128:]
    scs_np[:, 1, 0] = 1.0 + k_norm_scale[:128]
    scs_np[:, 1, 1] = 1.0 + k_norm_scale[128:]

    in_maps = []
    for core in range(8):
        b, tp = core // 4, core % 4
        sinu = positions[b].astype(np.float32)[:, None] / timescale[None, :]  # [T, 128]
        cos_np = np.cos(sinu).T.reshape(128, 4, 512).transpose(1, 0, 2)  # [4,128,512]
        sin_np = np.sin(sinu).T.reshape(128, 4, 512).transpose(1, 0, 2)
        # x^T packed [half, d, 128, 1024]
        xT = x[b].T.reshape(16, 128, 2, 1024).transpose(2, 0, 1, 3)
        # fused k/q weights [c, 128, 2, 3, 256]
        wA_np = np.empty((16, 128, 3, 256), np.float32)
        kw = k_w[tp]; qw = q_w[2 * tp:2 * tp + 2]
        wA_np[:, :, 0, :] = kw.reshape(16, 128, 256)
        wA_np[:, :, 1, :] = qw[0].reshape(16, 128, 256)
        wA_np[:, :, 2, :] = qw[1].reshape(16, 128, 256)
        wA_np = wA_np.reshape(8, 2, 128, 3, 256).transpose(0, 2, 1, 3, 4)
        ow_np = o_w[2 * tp:2 * tp + 2].reshape(2, 2, 128, D)  # [nl, hh, 128, D]
        in_maps.append({
            "xh": np.ascontiguousarray(xT).astype(NPB),
            "wA": np.ascontiguousarray(wA_np).astype(NPB),
            "vw": np.ascontiguousarray(
                v_w[tp].reshape(16, 128, 256).transpose(1, 0, 2)).astype(NPB),
            "ow": np.ascontiguousarray(ow_np).astype(NPB),
            "cosT": np.ascontiguousarray(cos_np),
            "sinT": np.ascontiguousarray(sin_np),
            "masks": masks_np,
            "scs": scs_np,
        })

    res = run_bass_kernel_spmd(nc, in_maps, core_ids=list(range(8)))
    last_results = res

    out = np.zeros((B, T, D), np.float32)
    for core in range(8):
        out[core // 4] += np.asarray(res.results[core]["out"]).astype(np.float32).reshape(T, D)
    return out
